# revision 18
# baseline (speedup 1.0000x reference)
import os
import sys
import types

import numpy as np

if "/opt/trn_rl_repo" not in sys.path and not any(
    p.endswith("trn_rl_repo") for p in sys.path
):
    sys.path.insert(0, "/opt/trn_rl_repo")

B = 16384
EMB = 512
NUM_COMMANDS = 4
NUM_MEAS = 8
NCORES = 8
P = 128

_CACHE = {}


def _install_ntff_shim():
    if "antenv.axon_hooks" in sys.modules:
        return
    try:
        import antenv

        mod = types.ModuleType("antenv.axon_hooks")
        mod._hook = None
        mod.set_axon_ntff_profile_hook = lambda h: setattr(mod, "_hook", h)
        mod.get_axon_ntff_profile_hook = lambda: mod._hook
        sys.modules["antenv.axon_hooks"] = mod
        antenv.axon_hooks = mod
        from trn_agent_boot.trn_boot import _ntff_profile_via_ctypes

        mod.set_axon_ntff_profile_hook(
            _ntff_profile_via_ctypes("/opt/axon/libaxon_pjrt.so")
        )
    except Exception:
        pass


def _split_excess_waits(nc, max_waits=1):
    from concourse import mybir

    n_split = 0
    for f in nc.m.functions:
        for bb in f.blocks:
            insts = list(bb.instructions)
            new_insts = []
            changed = False
            for inst in insts:
                si = inst.sync_info
                if si is not None and si.on_wait and len(si.on_wait) > max_waits:
                    waits = list(si.on_wait)
                    extra, keep = waits[:-max_waits], waits[-max_waits:]
                    while extra:
                        chunk, extra = extra[:max_waits], extra[max_waits:]
                        n_split += 1
                        nop = mybir.InstNoOp(
                            name=f"waitsplit_{n_split}_{inst.name}",
                            engine=inst.engine,
                            ins=[],
                            outs=[],
                            sync_info=mybir.SyncInfo(on_wait=chunk, on_update=[]),
                        )
                        new_insts.append(nop)
                    si.on_wait = keep
                    changed = True
                new_insts.append(inst)
            if changed:
                bb.instructions.clear()
                for i in new_insts:
                    bb.instructions.append(i)
    return n_split


def _strip_const_loads(nc):
    used = set()
    removed = 0
    for f in nc.m.functions:
        for bb in f.blocks:
            for inst in bb.instructions:
                for arg in list(inst.ins):
                    t = getattr(getattr(arg, "bass_ap", None), "tensor", None)
                    n = getattr(t, "name", "") or ""
                    if n.startswith("const-"):
                        used.add(n)
    if used:
        return 0
    for f in nc.m.functions:
        for bb in f.blocks:
            keep = []
            for inst in bb.instructions:
                if type(inst).__name__ == "InstTensorLoad":
                    outs = list(inst.outs)
                    names = []
                    for a in outs:
                        t = getattr(getattr(a, "bass_ap", None), "tensor", None)
                        names.append(getattr(t, "name", "") or "")
                    if names and all(n.startswith("const-") for n in names):
                        removed += 1
                        continue
                keep.append(inst)
            if len(keep) != len(bb.instructions):
                bb.instructions.clear()
                for i in keep:
                    bb.instructions.append(i)
    return removed


def _strip_tail(nc):
    from concourse import mybir

    f = nc.m.functions[0]
    bb = f.blocks[-1]
    insts = list(bb.instructions)
    idx = None
    for i, inst in enumerate(insts):
        if isinstance(inst, mybir.InstDrain) and inst.engine == mybir.EngineType.SP:
            idx = i
            break
    if idx is None:
        return 0
    kept = insts[: idx + 1]
    drain = kept[-1]
    if drain.sync_info is not None:
        drain.sync_info.on_wait = []
    removed = len(insts) - len(kept)
    bb.instructions.clear()
    for i in kept:
        bb.instructions.append(i)
    return removed


def _bf16():
    import ml_dtypes

    return ml_dtypes.bfloat16


def _route(command):
    idx_by_core = []
    for e in range(NUM_COMMANDS):
        idx = np.nonzero(command == e)[0].astype(np.int64)
        h = (len(idx) + 1) // 2
        idx_by_core.append(idx[:h])
        idx_by_core.append(idx[h:])
    cap = max((len(ix) for ix in idx_by_core), default=1)
    cap = max(cap, 1)
    rows = [len(ix) for ix in idx_by_core]
    I = np.stack(
        [
            np.concatenate(
                [ix, np.full(cap - len(ix), ix[-1] if len(ix) else 0, np.int64)]
            )
            for ix in idx_by_core
        ]
    )
    return cap, rows, I


def _group_sizes(cap):
    sizes = [512] * (cap // 512)
    if cap % 512:
        sizes.append(cap % 512)
    return sizes


def _build_program(cap, sizes):
    from contextlib import ExitStack

    import concourse.bass as bass
    import concourse.tile as tile
    from concourse import mybir

    f32 = mybir.dt.float32
    BF = mybir.dt.bfloat16
    G = len(sizes)
    offs = np.concatenate([[0], np.cumsum(sizes)]).astype(int)

    m1 = 1 + (G - 1) // 2
    pieces = [(0, offs[1]), (offs[1], offs[m1]), (offs[m1], cap)]
    pieces = [(a, b) for (a, b) in pieces if b > a]

    def piece_of(g):
        for pi, (a, b) in enumerate(pieces):
            if offs[g] >= a and offs[g + 1] <= b:
                return pi, a
        raise AssertionError

    MR = 32 * 3 + NUM_MEAS + 1
    nc = bass.Bass()
    img_d = nc.declare_dram_parameter("img_pre", [P, 4 * cap], BF, isOutput=False)
    meas_d = nc.declare_dram_parameter("meas_pre", [MR, cap], BF, isOutput=False)
    A_d = nc.declare_dram_parameter("A_pre", [P, 16 * P], BF, isOutput=False)
    Wf_d = nc.declare_dram_parameter("Wf_pre", [MR, P], BF, isOutput=False)
    s_d = nc.declare_dram_parameter("s_pre", [P, 2], BF, isOutput=False)
    outp_d = nc.declare_dram_parameter("outp", [1, 8 * 512], f32, isOutput=True)
    gth_d = nc.declare_dram_parameter("gth", [1, 8 * 512], BF, isOutput=True)

    with tile.TileContext(nc) as tc:
        with ExitStack() as ctx:
            const_pool = ctx.enter_context(tc.tile_pool(name="const", bufs=1))
            w_pool = ctx.enter_context(tc.tile_pool(name="w", bufs=1))
            img_pool = ctx.enter_context(tc.tile_pool(name="img", bufs=1))
            r_pool = ctx.enter_context(tc.tile_pool(name="r", bufs=8))
            s_pool = ctx.enter_context(tc.tile_pool(name="s", bufs=2))
            junk_pool = ctx.enter_context(tc.tile_pool(name="junk", bufs=1))
            out_pool = ctx.enter_context(tc.tile_pool(name="out", bufs=1))
            ps_pool = ctx.enter_context(tc.tile_pool(name="ps", bufs=6, space="PSUM"))
            pso_pool = ctx.enter_context(
                tc.tile_pool(name="pso", bufs=2, space="PSUM")
            )

            SY, SC, GP = nc.sync, nc.scalar, nc.gpsimd

            meas_sb = const_pool.tile([P, cap], BF, tag="meas", name="meas_sb")
            Wf_sb = const_pool.tile([P, P], BF, tag="wf", name="Wf_sb")
            s_sb = const_pool.tile([P, 2], BF, tag="s", name="s_sb")
            A_sb = {
                0: w_pool.tile([P, 8 * P], BF, tag="A0", name="A_sb0"),
                1: w_pool.tile([P, 8 * P], BF, tag="A1", name="A_sb1"),
            }
            img_sb = {}
            for d in range(4):
                for pi, (a, b) in enumerate(pieces):
                    img_sb[d, pi] = img_pool.tile(
                        [P, b - a], BF, tag=f"im{d}_{pi}", name=f"img_sb_{d}_{pi}"
                    )
            outs_sb = out_pool.tile([1, 8 * 512], f32, tag="outs", name="outs_sb")

            GP.dma_start(s_sb[:], s_d[:])
            SY.dma_start(meas_sb[:MR, :], meas_d[:])
            SC.dma_start(Wf_sb[:MR, :], Wf_d[:])
            SY.dma_start(A_sb[0][:], A_d[:, : 8 * P])
            SC.dma_start(A_sb[1][:], A_d[:, 8 * P :])
            a0, b0 = pieces[0]
            for d in range(4):
                q = SY if d % 2 == 0 else SC
                q.dma_start(
                    img_sb[d, 0][:], img_d[:, d * cap + a0 : d * cap + b0]
                )
            rr = [SY, SC, GP]
            ri = 0
            for pi in range(1, len(pieces)):
                a, b = pieces[pi]
                for d in range(4):
                    rr[ri % 3].dma_start(
                        img_sb[d, pi][:], img_d[:, d * cap + a : d * cap + b]
                    )
                    ri += 1

            warm_a = const_pool.tile([P, EMB], BF, tag="warm_a", name="warm_a")
            nc.vector.memset(warm_a[:], 0.0)
            ones_sb = const_pool.tile([P, 1], BF, tag="ones", name="ones_sb")
            nc.vector.memset(ones_sb[:], 1.0)
            zbias = const_pool.tile([P, 1], f32, tag="zb", name="zbias")
            nc.vector.memset(zbias[:], 0.0)
            ps_w = ps_pool.tile([P, EMB], f32, tag="h", name="ps_warm")
            N_WARM = 12
            for w in range(N_WARM):
                nc.tensor.matmul(
                    ps_w[:, :P],
                    lhsT=warm_a[:, :P],
                    rhs=warm_a[:, :P],
                    start=(w == 0),
                    stop=(w == N_WARM - 1),
                )
            junkw = junk_pool.tile([P, 1], f32, tag="junk")
            nc.vector.tensor_scalar(
                junkw[:],
                ps_w[:, :1],
                0.0,
                None,
                mybir.AluOpType.add,
            )

            red = {}
            out_ps_of = {}

            def emit_reduction(g, which):
                r01, r23, ng = red[g]
                rsrc = r01 if which == 0 else r23
                out_ps = out_ps_of[g]
                nc.tensor.matmul(
                    out_ps[:, :ng],
                    lhsT=s_sb[:, which : which + 1],
                    rhs=rsrc[:, :ng],
                    start=(which == 0),
                    stop=(which == 1),
                )
                if which == 1:
                    nc.vector.tensor_scalar(
                        outs_sb[:, 512 * g : 512 * g + ng],
                        out_ps[:, :ng],
                        0.0,
                        None,
                        mybir.AluOpType.add,
                    )

            for g in range(G):
                ng = sizes[g]
                off = offs[g]
                pi, pa = piece_of(g)
                poff = off - pa
                out_ps_of[g] = pso_pool.tile(
                    [1, 512], f32, tag="out_ps", name=f"op_{g}"
                )
                ps = {}
                for j in range(4):
                    ps[j] = ps_pool.tile([P, 512], f32, tag="h", name=f"ps_{g}_{j}")
                    nc.tensor.matmul(
                        ps[j][:, :ng],
                        lhsT=Wf_sb[32 * j : 32 * j + NUM_MEAS + 1, :],
                        rhs=meas_sb[32 * j : 32 * j + NUM_MEAS + 1, off : off + ng],
                        start=True,
                        stop=False,
                        tile_position=(32 * j, 0),
                    )
                if g > 0:
                    emit_reduction(g - 1, 0)
                r = {}
                r01 = s_pool.tile([P, 512], BF, tag="r01", name=f"r01_{g}")
                r23 = s_pool.tile([P, 512], BF, tag="r23", name=f"r23_{g}")
                for j in range(4):
                    for d in range(4):
                        blk = 4 * (j % 2) + d
                        nc.tensor.matmul(
                            ps[j][:, :ng],
                            lhsT=A_sb[j // 2][:, blk * P : (blk + 1) * P],
                            rhs=img_sb[d, pi][:, poff : poff + ng],
                            start=False,
                            stop=(d == 3),
                        )
                    r[j] = r_pool.tile([P, 512], BF, tag="r", name=f"r_{g}_{j}")
                    nc.scalar.activation(
                        r[j][:, :ng],
                        ps[j][:, :ng],
                        mybir.ActivationFunctionType.Relu,
                        bias=zbias[:],
                    )
                    if j == 1:
                        nc.gpsimd.tensor_tensor(
                            r01[:, :ng],
                            r[0][:, :ng],
                            r[1][:, :ng],
                            mybir.AluOpType.add,
                        )
                    if j == 2 and g > 0:
                        emit_reduction(g - 1, 1)
                    if j == 3:
                        nc.gpsimd.tensor_tensor(
                            r23[:, :ng],
                            r[2][:, :ng],
                            r[3][:, :ng],
                            mybir.AluOpType.add,
                        )
                        SY.dma_start(
                            gth_d[:, 512 * g : 512 * g + ng],
                            r[3][96:97, :ng],
                        )
                red[g] = (r01, r23, ng)
            emit_reduction(G - 1, 0)
            emit_reduction(G - 1, 1)
            SY.dma_start(outp_d[:, : 512 * G], outs_sb[:, : 512 * G])

    _strip_tail(nc)
    if os.environ.get("KERNEL_STRIP_CONST", "1") == "1":
        _strip_const_loads(nc)
    _split_excess_waits(nc)
    return nc


def _pair_sort(w2col):
    pos = list(np.nonzero(w2col > 0)[0])
    neg = list(np.nonzero(w2col <= 0)[0])
    gamma = 0.0
    mixed = None
    if len(pos) % 2 == 1:
        mixed = (pos.pop(), neg.pop())
        gamma = 2.0
    pairs = [(pos[i], pos[i + 1], 1.0) for i in range(0, len(pos), 2)]
    pairs += [(neg[i], neg[i + 1], -1.0) for i in range(0, len(neg), 2)]
    assert len(pairs) == (255 if mixed else 256)
    pairsA = pairs[:128]
    pairsB = pairs[128:]
    if mixed is not None:
        pairsB.insert(96, (mixed[0], mixed[1], 1.0))
    perm = np.empty(512, np.int64)
    sA = np.empty(128, np.float32)
    sB = np.empty(128, np.float32)
    for p in range(128):
        a0, a1, sa = pairsA[p]
        b0, b1, sb = pairsB[p]
        perm[p] = a0
        perm[128 + p] = a1
        perm[256 + p] = b0
        perm[384 + p] = b1
        sA[p] = sa
        sB[p] = sb
    return perm, sA, sB, gamma


def _prepare(inputs):
    bf16 = _bf16()
    img_embs = np.asarray(inputs["img_embs"], np.float32)
    measurements = np.asarray(inputs["measurements"], np.float32)
    command = np.asarray(inputs["command"])
    W_meas = np.asarray(inputs["W_meas"], np.float32)
    b_meas = np.asarray(inputs["b_meas"], np.float32)
    W1 = np.asarray(inputs["W1"], np.float32)
    b1 = np.asarray(inputs["b1"], np.float32)
    W2 = np.asarray(inputs["W2"], np.float32)
    b2 = np.asarray(inputs["b2"], np.float32)

    cap, rows, I = _route(command)
    sizes = _group_sizes(cap)

    W1h = W1[:, EMB:, :].astype(np.float64)
    Wf = np.einsum("md,edh->emh", W_meas.astype(np.float64), W1h)
    b_eff = np.einsum("d,edh->eh", b_meas.astype(np.float64), W1h) + b1

    imgT = np.ascontiguousarray(img_embs.T).astype(bf16)
    measT = measurements.T
    ones_row = np.ones((1, cap), np.float32)

    MR = 32 * 3 + NUM_MEAS + 1
    in_maps = []
    b2c = [float(x) for x in b2[:, 0]]
    gammas = []
    ew = {}
    for e in range(NUM_COMMANDS):
        w2col = W2[e][:, 0].astype(np.float64)
        perm, sA, sB, gamma = _pair_sort(w2col)
        absw2 = np.abs(w2col)[perm]
        A_eff = W1[e][:EMB, :].astype(np.float64)[:, perm] * absw2[None, :]
        A_pre = np.ascontiguousarray(
            A_eff.reshape(EMB, 4, P).transpose(0, 2, 1).reshape(1, -1)
        )
        A_pre = np.ascontiguousarray(
            A_eff.reshape(4, P, 4, P).transpose(1, 2, 0, 3).reshape(P, 16 * P)
        ).astype(bf16)
        WfAug = np.concatenate([Wf[e], b_eff[e][None, :]], axis=0)
        WfAug = WfAug[:, perm] * absw2[None, :]
        Wf_blk = WfAug.reshape(NUM_MEAS + 1, 4, P).transpose(1, 0, 2)
        Wf_pre = np.zeros((MR, P), bf16)
        meas_mask = slice(None)
        for j in range(4):
            Wf_pre[32 * j : 32 * j + NUM_MEAS + 1, :] = Wf_blk[j].astype(bf16)
        s_pre = np.stack([sA, sB], axis=1).astype(bf16)
        ew[e] = (A_pre, Wf_pre, s_pre, gamma)

    for k in range(NCORES):
        e = k // 2
        Ik = I[k]
        img_pre = np.ascontiguousarray(
            imgT[:, Ik].reshape(4, P, cap).transpose(1, 0, 2).reshape(P, 4 * cap)
        )
        measAug = np.concatenate([measT[:, Ik], ones_row], axis=0).astype(bf16)
        meas_pre = np.zeros((MR, cap), bf16)
        for j in range(4):
            meas_pre[32 * j : 32 * j + NUM_MEAS + 1, :] = measAug
        A_pre, Wf_pre, s_pre, gamma = ew[e]
        gammas.append(gamma)
        in_maps.append(
            {
                "img_pre": img_pre,
                "meas_pre": meas_pre,
                "A_pre": A_pre,
                "Wf_pre": Wf_pre,
                "s_pre": s_pre,
            }
        )
    return in_maps, I, rows, cap, sizes, b2c, gammas


def _run(inputs, trace=False):
    _install_ntff_shim()
    from concourse.bass_utils import run_bass_kernel_spmd

    in_maps, I, rows, cap, sizes, b2c, gammas = _prepare(inputs)
    key = (cap, tuple(sizes))
    if key not in _CACHE:
        _CACHE[key] = _build_program(cap, sizes)
    nc = _CACHE[key]

    res = run_bass_kernel_spmd(
        nc, in_maps, core_ids=list(range(NCORES)), trace=trace
    )

    G = len(sizes)
    nb = int(np.asarray(inputs["command"]).shape[0])
    angle = np.zeros(nb, np.float32)
    speed = np.zeros(nb, np.float32)
    for k in range(NCORES):
        if rows[k] == 0:
            continue
        outp = res.results[k]["outp"].reshape(8, 512)
        gth = res.results[k]["gth"].reshape(8, 512).astype(np.float32)
        p = np.concatenate(
            [
                outp[g, : sizes[g]] - np.float32(gammas[k]) * gth[g, : sizes[g]]
                for g in range(G)
            ]
        )[: rows[k]]
        p = p + np.float32(b2c[k // 2])
        Ik = I[k][: rows[k]]
        angle[Ik] = (1.0 / (1.0 + np.exp(-p, dtype=np.float32))) * np.float32(50.0)
        speed[Ik] = np.clip(p, -1.0, 1.0)
    return (angle, speed), res


def kernel(**inputs):
    out, _ = _run(inputs)
    return out
